# revision 77
# baseline (speedup 1.0000x reference)
"""Trainium2 Bass kernel for nn_BRepFaceEncoder (gnn_message_passing).

Sharding: the 60000 faces are split contiguously across 8 NeuronCores. Each
core back-chains the halo it needs (faces -> loops -> edges -> vertices) and
runs the whole pipeline locally - no collectives.

Math identities used:
  segment_max_d(x_dst[d] - x_src[s]) == x_dst[d] - segment_min_s(x_src[s])
  min(leaky(z)) == leaky(min(z))   (monotone; exact - conv1 only)
  concat([x, x - m]) @ Wc == x @ (A + B) + m @ (-B)   (A=Wc[:H], B=Wc[H:])

All compute in bf16 (PE matmul 1 cyc/row vs 4 for fp32; DVE 2x on 16-bit),
fp32 PSUM accumulation. conv1 needs no gather: raw vertex positions are
host-staged into per-round slot order and min-accumulated in pre-activation
space. conv2/3 gather previous-layer rows with per-column gpsimd indirect
DMAs (one row-index per partition per op), folded with DVE min trees.
Destinations are degree-sorted into 128-row blocks bucketed by round count R.

Leaky runs as a single native Lrelu op on the Act engine (alpha=0.01,
HW-verified); conv1's min accumulates directly in transposed space via
wv-chunk-as-lhsT matmuls, so no PE transposes or bridge copies are needed
there. Engine assignment is balanced so PE (~530us busy) stays the limiter.
Gathers are per-column: multi-index SWDGE offsets mis-read on real hardware
(p>=64 odd slots; W>4 reads offsets 64-bit-strided), single-column is exact.
"""

import sys
from contextlib import ExitStack

import numpy as np
import ml_dtypes

if "/opt/trn_rl_repo" not in sys.path:
    sys.path.insert(0, "/opt/trn_rl_repo")

import concourse.bass as bass            # noqa: E402
import concourse.tile as tile            # noqa: E402
from concourse import bacc, mybir        # noqa: E402
from concourse.bass_utils import run_bass_kernel_spmd  # noqa: E402
from concourse.masks import make_identity              # noqa: E402

f32 = mybir.dt.float32
bf16 = mybir.dt.bfloat16
i32 = mybir.dt.int32
ALU = mybir.AluOpType
NPBF = ml_dtypes.bfloat16

H = 256
C = 8
BIG = np.float32(512.0)
BUCKETS = (1, 2, 3, 4, 5, 6, 7, 8, 9, 10, 12, 14, 16, 20, 24, 32)
GMAP = {1: 4, 2: 4, 3: 4, 4: 4, 5: 2, 6: 2, 7: 2, 8: 2, 9: 1, 10: 1,
        12: 1, 14: 1, 16: 1, 20: 1, 24: 1, 32: 1}


# ==========================================================================
# Host-side schedule construction
# ==========================================================================

def _build_conv_schedule(dloc, sloc, n_dst):
    """Blocks of 128 degree-sorted dsts, bucketed by round count R."""
    counts = np.bincount(dloc, minlength=n_dst)
    order_p = np.argsort(dloc, kind="stable")
    srcs_sorted = sloc[order_p]
    starts = np.zeros(n_dst + 1, dtype=np.int64)
    np.cumsum(counts, out=starts[1:])

    perm = np.argsort(-counts, kind="stable")
    n_blk = (n_dst + 127) // 128
    pad = n_blk * 128 - n_dst
    perm_padded = np.concatenate([perm, np.full(pad, perm[-1] if n_dst else 0,
                                                dtype=perm.dtype)])
    deg_padded = counts[perm_padded]
    deg_padded[n_dst:] = 0

    bucket_blocks = {}
    for b in range(n_blk):
        dsts = perm_padded[b * 128:(b + 1) * 128]
        degs = deg_padded[b * 128:(b + 1) * 128]
        mx = int(degs[0])
        R = next(r for r in BUCKETS if r >= max(mx, 1))
        slots = np.full((128, R), -1, dtype=np.int64)
        base = starts[dsts]
        for r in range(R):
            have = degs > r
            if have.any():
                slots[have, r] = srcs_sorted[base[have] + r]
        d, s = bucket_blocks.setdefault(R, ([], []))
        d.append(dsts)
        s.append(slots)
    return {R: (np.stack(d), np.stack(s)) for R, (d, s) in bucket_blocks.items()}


def _host_prep(inputs):
    e2v = np.asarray(inputs["edge_to_vertex"])
    l2e = np.asarray(inputs["loop_to_edge"])
    f2l = np.asarray(inputs["face_to_loop"])

    NV = inputs["vertex_positions"].shape[0]
    NE = inputs["edge_curves"].shape[0]
    NL = inputs["loop_types"].shape[0]
    NF = inputs["face_surfaces"].shape[0]

    pos = np.asarray(inputs["vertex_positions"], np.float32)
    raw_feats = [
        np.concatenate([np.asarray(inputs["edge_curves"], np.float32),
                        np.asarray(inputs["edge_curve_parameters"], np.float32),
                        np.asarray(inputs["edge_curve_flipped"], np.float32)[:, None]], axis=1),
        np.asarray(inputs["loop_types"], np.float32),
        np.concatenate([np.asarray(inputs["face_surfaces"], np.float32),
                        np.asarray(inputs["face_surface_parameters"], np.float32),
                        np.asarray(inputs["face_surface_flipped"], np.float32)[:, None]], axis=1),
    ]

    cores = []
    for i in range(C):
        lo, hi = i * NF // C, (i + 1) * NF // C
        mask = np.zeros(NF, bool); mask[lo:hi] = True
        m3 = mask[f2l[0]]
        d3, s3 = f2l[0][m3] - lo, f2l[1][m3]
        loops_i = np.unique(s3)
        mask = np.zeros(NL, bool); mask[loops_i] = True
        m2 = mask[l2e[0]]
        d2, s2 = l2e[0][m2], l2e[1][m2]
        edges_i = np.unique(s2)
        mask = np.zeros(NE, bool); mask[edges_i] = True
        m1 = mask[e2v[0]]
        d1, s1 = e2v[0][m1], e2v[1][m1]
        verts_i = np.unique(s1)

        sch = [
            _build_conv_schedule(np.searchsorted(edges_i, d1),
                                 np.searchsorted(verts_i, s1), len(edges_i)),
            _build_conv_schedule(np.searchsorted(loops_i, d2),
                                 np.searchsorted(edges_i, s2), len(loops_i)),
            _build_conv_schedule(d3, np.searchsorted(loops_i, s3), hi - lo),
        ]
        cores.append(dict(lo=lo, hi=hi, loops=loops_i, edges=edges_i,
                          verts=verts_i, sch=sch))

    # global padded sizes
    NVp = ((max(len(c["verts"]) for c in cores) + 511) // 512) * 512
    bucket_counts = [{}, {}, {}]
    for k in range(3):
        for c in cores:
            for R, (d, s) in c["sch"][k].items():
                g = GMAP[R]
                n = -(-d.shape[0] // g) * g
                bucket_counts[k][R] = max(bucket_counts[k].get(R, 0), n)
    nblk = [sum(bucket_counts[k].values()) for k in range(3)]

    # gather-source tables (rows of data; +1 dummy row appended on device)
    tab_rows = [NVp, nblk[0] * 128, nblk[1] * 128]

    meta = dict(NVp=NVp, bucket_counts=bucket_counts, nblk=nblk,
                tab_rows=tab_rows, F=[16, 11, 18])

    per_core_inputs = []
    per_core_rowmaps = []
    for c in cores:
        im = {}
        nvl = len(c["verts"])
        pT = np.zeros((4, NVp), np.float32)
        pT[:3, :nvl] = pos[c["verts"]].T
        pT[3, :] = 1.0

        ent_ids = [c["edges"], c["loops"], np.arange(c["lo"], c["hi"]) ]
        prev_rowmap = None   # maps local src entity id -> previous table row
        rowmaps = []
        for k in range(3):
            n_dst = len(ent_ids[k])
            raws = raw_feats[k][ent_ids[k]]
            Fk = raws.shape[1] + 1
            rawT = np.empty((Fk, nblk[k] * 128), np.float32)
            rawT[:-1, :] = raws[0][:, None]
            rawT[-1, :] = 1.0
            rowmap = np.zeros(n_dst, np.int64)

            src_rows = meta["tab_rows"][k]
            dummy = src_rows  # dummy row index in source table

            row_base = 0
            for R in BUCKETS:
                nb = bucket_counts[k].get(R, 0)
                if nb == 0:
                    continue
                g = GMAP[R]
                W = g * R
                if k == 0:
                    slot_buf = np.zeros((nb // g, 4, W * 128), np.float32)
                else:
                    idx_buf = np.full((128, (nb // g) * W), dummy,
                                      np.int32)
                if R in c["sch"][k]:
                    d_all, s_all = c["sch"][k][R]
                else:
                    d_all = np.zeros((0, 128), np.int64)
                    s_all = np.zeros((0, 128, R), np.int64)
                nb_real = d_all.shape[0]
                # rawT columns + rowmap for real blocks
                if nb_real:
                    rows = row_base + np.arange(nb_real * 128)
                    dflat = d_all.reshape(-1)
                    rawT[:-1, rows] = raws[dflat].T
                    # rowmap: first assignment wins for duplicated pad dsts;
                    # real dsts appear exactly once among non-pad positions.
                    rowmap[dflat[::-1]] = rows[::-1]
                # slots -> source-table rows
                for gi in range(nb // g):
                    for ci2 in range(g):
                        b = gi * g + ci2
                        if b >= nb_real:
                            continue
                        sl = s_all[b]              # [128, R] local src ids
                        mrow = sl >= 0
                        if k == 0:
                            # pads duplicate the first slot (min-neutral)
                            conv = np.where(mrow, sl, sl[:, :1])
                            for r in range(R):
                                w = r * g + ci2
                                slot_buf[gi, :, w * 128:(w + 1) * 128] = \
                                    pT[:, conv[:, r]]
                        else:
                            conv = np.full_like(sl, dummy)
                            conv[mrow] = prev_rowmap[sl[mrow]]
                            for r in range(R):
                                idx_buf[:, gi * W + r * g + ci2] = conv[:, r]
                if k == 0:
                    im[f"pslot{R}"] = slot_buf.astype(NPBF)
                else:
                    im[f"idx{k}_{R}"] = idx_buf
                row_base += nb * 128
            im[f"rawT{k}"] = rawT.astype(NPBF)
            rowmaps.append(rowmap)
            prev_rowmap = rowmap
        per_core_inputs.append(im)
        per_core_rowmaps.append(rowmaps)

    # weights (identical on every core)
    def _lin_w(W_, b_):
        return np.concatenate([np.asarray(W_, np.float32),
                               np.asarray(b_, np.float32)[None]], 0).astype(NPBF)

    wshared = {
        "wv": _lin_w(inputs["Wv"], inputs["bv"]),
        "wx0": _lin_w(inputs["We"], inputs["be"]),
        "wx1": _lin_w(inputs["Wl"], inputs["bl"]),
        "wx2": _lin_w(inputs["Wf"], inputs["bf"]),
    }
    for k, (wn, bn) in enumerate([("Wve", "bve"), ("Wel", "bel"), ("Wlf", "blf")]):
        Wc = np.asarray(inputs[wn], np.float32)
        A, B = Wc[:H], Wc[H:]
        wshared[f"wS{k}"] = (A + B).astype(NPBF)
        wshared[f"wnB{k}"] = (-B).astype(NPBF)
        bc = np.asarray(inputs[bn], np.float32)[None]
        wshared[f"bcr{k}"] = np.tile(bc, (1, 4)).astype(NPBF)
    for im in per_core_inputs:
        im.update(wshared)

    return meta, per_core_inputs, per_core_rowmaps, cores


# ==========================================================================
# Device kernel builder
# ==========================================================================

def _build_kernel(meta, rep=1, use_lrelu=True):
    nblk = meta["nblk"]
    Fs = meta["F"]

    nc = bacc.Bacc("TRN2", target_bir_lowering=False, debug=False,
                   num_devices=C)

    t_rawT = [nc.dram_tensor(f"rawT{k}", [Fs[k], nblk[k] * 128], bf16,
                             kind="ExternalInput") for k in range(3)]
    t_idx = {}
    t_pslot = {}
    for k in range(3):
        for R in BUCKETS:
            nb = meta["bucket_counts"][k].get(R, 0)
            if nb == 0:
                continue
            g = GMAP[R]
            if k == 0:
                t_pslot[R] = nc.dram_tensor(
                    f"pslot{R}", [nb // g, 4, g * R * 128], bf16,
                    kind="ExternalInput")
            else:
                t_idx[(k, R)] = nc.dram_tensor(
                    f"idx{k}_{R}", [128, (nb // g) * g * R], i32,
                    kind="ExternalInput")
    t_wv = nc.dram_tensor("wv", [4, H], bf16, kind="ExternalInput")
    t_wx = [nc.dram_tensor(f"wx{k}", [Fs[k], H], bf16, kind="ExternalInput")
            for k in range(3)]
    t_wS = [nc.dram_tensor(f"wS{k}", [H, H], bf16, kind="ExternalInput")
            for k in range(3)]
    t_wnB = [nc.dram_tensor(f"wnB{k}", [H, H], bf16, kind="ExternalInput")
             for k in range(3)]
    t_bcr = [nc.dram_tensor(f"bcr{k}", [1, 4 * H], bf16, kind="ExternalInput")
             for k in range(3)]

    # tables: data rows + 1 dummy(+BIG) row
    t_e1 = nc.dram_tensor("e1", [nblk[0] * 128 + 1, H], bf16, kind="Internal")
    t_l2 = nc.dram_tensor("l2", [nblk[1] * 128 + 1, H], bf16, kind="Internal")
    t_f3 = nc.dram_tensor("f3", [nblk[2] * 128, H], f32, kind="ExternalOutput")
    tables = [None, t_e1, t_l2]
    outs = [t_e1, t_l2, t_f3]

    with tile.TileContext(nc) as tc, ExitStack() as ctx:
        const = ctx.enter_context(tc.tile_pool(name="const", bufs=1))
        pgath = ctx.enter_context(tc.tile_pool(name="pgath", bufs=6))
        pps = ctx.enter_context(tc.tile_pool(name="pps", bufs=3))
        praw = ctx.enter_context(tc.tile_pool(name="praw", bufs=3))
        pxT = ctx.enter_context(tc.tile_pool(name="pxT", bufs=10))
        pmT = ctx.enter_context(tc.tile_pool(name="pmT", bufs=8))
        pbr = ctx.enter_context(tc.tile_pool(name="pbr", bufs=6))
        pmta = ctx.enter_context(tc.tile_pool(name="pmta", bufs=6))
        ptmp = ctx.enter_context(tc.tile_pool(name="ptmp", bufs=10))
        pout = ctx.enter_context(tc.tile_pool(name="pout", bufs=4))
        psA = ctx.enter_context(tc.tile_pool(name="psA", bufs=4, space="PSUM"))
        psT = ctx.enter_context(tc.tile_pool(name="psT", bufs=2, space="PSUM"))
        psO = ctx.enter_context(tc.tile_pool(name="psO", bufs=2, space="PSUM"))

        ident = const.tile([128, 128], bf16)
        make_identity(nc, ident[:])
        ones_row = const.tile([1, 128], bf16)
        nc.vector.memset(ones_row[:], 1.0)

        # resident weights
        wv_sb = const.tile([4, H], bf16)
        nc.scalar.dma_start(out=wv_sb[:], in_=t_wv.ap()[:, :])
        wx_sb = []
        for k in range(3):
            t = const.tile([Fs[k], H], bf16, tag=f"wx{k}")
            nc.scalar.dma_start(out=t[:], in_=t_wx[k].ap()[:, :])
            wx_sb.append(t)
        wS_sb, wnB_sb = [], []
        for k in range(3):
            cs_, cb_ = [], []
            for j in range(2):
                t = const.tile([128, H], bf16, tag=f"wS{k}_{j}")
                nc.scalar.dma_start(out=t[:], in_=t_wS[k].ap()[j * 128:(j + 1) * 128, :])
                cs_.append(t)
                t = const.tile([128, H], bf16, tag=f"wnB{k}_{j}")
                nc.scalar.dma_start(out=t[:], in_=t_wnB[k].ap()[j * 128:(j + 1) * 128, :])
                cb_.append(t)
            wS_sb.append(cs_)
            wnB_sb.append(cb_)
        bcr_sb = []
        for k in range(3):
            b1 = const.tile([1, 4 * H], bf16, tag=f"bcr{k}")
            nc.scalar.dma_start(out=b1[:], in_=t_bcr[k].ap()[:, :])
            bcr_sb.append(b1)

        # dummy (+BIG) rows for the gather tables
        bigt = const.tile([1, H], bf16)
        nc.vector.memset(bigt[:], float(BIG))
        nc.sync.dma_start(out=t_e1.ap()[nblk[0] * 128:, :], in_=bigt[:])
        nc.sync.dma_start(out=t_l2.ap()[nblk[1] * 128:, :], in_=bigt[:])

        # resident gather indices, one tile per (k, R)
        idx_sb = {}
        for (k, R), t in t_idx.items():
            g = GMAP[R]
            W = g * R
            ngr = meta["bucket_counts"][k][R] // g
            it = const.tile([128, ngr * W], i32, tag=f"idxall{k}_{R}")
            nc.sync.dma_start(out=it[:], in_=t_idx[(k, R)].ap()[:, :])
            idx_sb[(k, R)] = it

        def leaky(out_ap, in_ap, ncols):
            """leaky(z) = max(z, 0.01 z); native Lrelu on Act when enabled."""
            if use_lrelu:
                nc.scalar.activation(out=out_ap, in_=in_ap,
                                     func=mybir.ActivationFunctionType.Lrelu,
                                     alpha=0.01)
            else:
                tt = ptmp.tile([128, 2 * H], bf16, tag="ttl")
                nc.scalar.mul(out=tt[:, :ncols], in_=in_ap, mul=0.01)
                nc.vector.tensor_tensor(out=out_ap, in0=in_ap,
                                        in1=tt[:, :ncols], op=ALU.max)

        def do_group(k, R, gi, pst, rawl):
            """One g-group: 128g dsts of conv k, bucket R."""
            g = GMAP[R]
            W = g * R
            out_t = outs[k]
            wx = wx_sb[k]
            row_base = row_bases[(k, R)]
            base = row_base + gi * g * 128

            if k == 0:
                # transposed min-accumulate: zT = wv_chunk.T @ posSlot lands
                # [h-dims, slots] directly, so no PE transposes and the leaky
                # runs on SBUF (gpsimd) with no bridge copy.
                mx = []
                for h2 in range(2):
                    if R == 1:
                        # single source: leaky straight off PSUM, no staging
                        zTr = psA.tile([128, 512], f32, tag="psA")
                        nc.tensor.matmul(
                            out=zTr[:, :g * 128],
                            lhsT=wv_sb[:, h2 * 128:(h2 + 1) * 128],
                            rhs=pst[:, :g * 128],
                            start=True, stop=True)
                        ttm = pbr.tile([128, g * 128], bf16, tag="ttm")
                        nc.scalar.mul(out=ttm[:], in_=zTr[:, :g * 128],
                                      mul=0.01)
                        mm = pmT.tile([128, g * 128], bf16, tag="mT")
                        nc.vector.tensor_tensor(out=mm[:],
                                                in0=zTr[:, :g * 128],
                                                in1=ttm[:], op=ALU.max)
                        mx.append(mm)
                        continue
                    mta = pmta.tile([128, 512], bf16, tag="mTacc")
                    for r in range(R):
                        zTr = psA.tile([128, 512], f32, tag="psA")
                        nc.tensor.matmul(
                            out=zTr[:, :g * 128],
                            lhsT=wv_sb[:, h2 * 128:(h2 + 1) * 128],
                            rhs=pst[:, r * g * 128:(r + 1) * g * 128],
                            start=True, stop=True)
                        if r == 0:
                            nc.scalar.copy(out=mta[:, :g * 128],
                                           in_=zTr[:, :g * 128])
                        else:
                            nc.vector.tensor_tensor(out=mta[:, :g * 128],
                                                    in0=mta[:, :g * 128],
                                                    in1=zTr[:, :g * 128],
                                                    op=ALU.min)
                    # leaky on DVE (SBUF bf16 so the max runs in 2x mode)
                    ttm = pbr.tile([128, g * 128], bf16, tag="ttm")
                    nc.vector.tensor_scalar(out=ttm[:], in0=mta[:, :g * 128],
                                            scalar1=0.01, scalar2=None,
                                            op0=ALU.mult)
                    mm = pmT.tile([128, g * 128], bf16, tag="mT")
                    nc.vector.tensor_tensor(out=mm[:], in0=mta[:, :g * 128],
                                            in1=ttm[:], op=ALU.max)
                    mx.append(mm)
            else:
                # per-column indirect gathers (multi-index offsets hit a
                # SWDGE firmware bug; single-column is HW-verified)
                gt = pgath.tile([128, W * H], bf16, tag="g")
                idx_all = idx_sb[(k, R)]
                for w in range(W):
                    nc.gpsimd.indirect_dma_start(
                        out=gt[:, w * H:(w + 1) * H],
                        out_offset=None,
                        in_=tables[k].ap()[:, :],
                        in_offset=bass.IndirectOffsetOnAxis(
                            ap=idx_all[:, gi * W + w:gi * W + w + 1], axis=0))
                # min fold over rounds (col w = r*g + c keeps block assoc)
                s = R
                while s > 1:
                    h = s // 2
                    nc.vector.tensor_tensor(
                        out=gt[:, : h * g * H], in0=gt[:, : h * g * H],
                        in1=gt[:, h * g * H: 2 * h * g * H], op=ALU.min)
                    if s % 2:
                        nc.vector.tensor_tensor(
                            out=gt[:, : g * H], in0=gt[:, : g * H],
                            in1=gt[:, (s - 1) * g * H: s * g * H],
                            op=ALU.min)
                    s = h

            # x_dst transposed halves: xT = leaky(wx.T @ rawl)
            xT = []
            for h2 in range(2):
                zT = psA.tile([128, g * 128], f32, tag="psA")
                nc.tensor.matmul(out=zT[:],
                                 lhsT=wx[:, h2 * 128:(h2 + 1) * 128],
                                 rhs=rawl[:], start=True, stop=True)
                xs = pxT.tile([128, g * 128], bf16, tag="xT")
                leaky(xs[:], zT[:], g * 128)
                xT.append(xs)

            if k > 0:
                # mT via PE transpose of min columns (both halves one tile)
                zmT = psT.tile([128, 4 * H], bf16, tag="psT")
                for h2 in range(2):
                    for ci2 in range(g):
                        base_m = ci2 * H
                        nc.tensor.transpose(
                            out=zmT[:, (h2 * g + ci2) * 128:(h2 * g + ci2 + 1) * 128],
                            in_=gt[:, base_m + h2 * 128: base_m + (h2 + 1) * 128],
                            identity=ident[:])
                mmb = pmT.tile([128, 2 * g * 128], bf16, tag="mT")
                nc.vector.tensor_scalar(out=mmb[:], in0=zmT[:, :2 * g * 128],
                                        scalar1=1.0, scalar2=None,
                                        op0=ALU.mult)
                mx = [mmb[:, :g * 128], mmb[:, g * 128:2 * g * 128]]

            # x row-major via bf16 PE transpose of xT (independent of zo)
            px = psT.tile([128, 4 * H], bf16, tag="psT")
            for ci2 in range(g):
                for h2 in range(2):
                    nc.tensor.transpose(
                        out=px[:, ci2 * H + h2 * 128: ci2 * H + (h2 + 1) * 128],
                        in_=xT[h2][:, ci2 * 128:(ci2 + 1) * 128],
                        identity=ident[:])

            out_sb = pout.tile([128, g * H], f32 if k == 2 else bf16, tag="outC")
            for p2 in range(0, g, 2):
                pw = min(2, g - p2)
                # zo = x@S + m@(-B) + bc
                zo = psO.tile([128, 2 * H], f32, tag="psO")
                for c2 in range(pw):
                    ci2 = p2 + c2
                    cs = slice(ci2 * 128, (ci2 + 1) * 128)
                    zr = zo[:, c2 * H:(c2 + 1) * H]
                    nc.tensor.matmul(out=zr, lhsT=ones_row[:],
                                     rhs=bcr_sb[k][:, :H], start=True, stop=False)
                    nc.tensor.matmul(out=zr, lhsT=xT[0][:, cs], rhs=wS_sb[k][0][:],
                                     start=False, stop=False)
                    nc.tensor.matmul(out=zr, lhsT=xT[1][:, cs], rhs=wS_sb[k][1][:],
                                     start=False, stop=False)
                    nc.tensor.matmul(out=zr, lhsT=mx[0][:, cs], rhs=wnB_sb[k][0][:],
                                     start=False, stop=False)
                    nc.tensor.matmul(out=zr, lhsT=mx[1][:, cs], rhs=wnB_sb[k][1][:],
                                     start=False, stop=True)

                # out = x + leaky(zo)
                lk = ptmp.tile([128, 2 * H], bf16, tag="lk")
                leaky(lk[:, :pw * H], zo[:, :pw * H], pw * H)
                nc.vector.tensor_tensor(out=out_sb[:, p2 * H:(p2 + pw) * H],
                                        in0=px[:, p2 * H:(p2 + pw) * H],
                                        in1=lk[:, :pw * H],
                                        op=ALU.add)
            nc.sync.dma_start(
                out=out_t.ap()[base:base + g * 128, :]
                    .rearrange("(c p) d -> p c d", p=128),
                in_=out_sb[:].rearrange("p (c d) -> p c d", d=H))

        # row base offsets per (k, R)
        row_bases = {}
        for k in range(3):
            rb = 0
            for R in BUCKETS:
                nb = meta["bucket_counts"][k].get(R, 0)
                if nb == 0:
                    continue
                row_bases[(k, R)] = rb
                rb += nb * 128

        for _rep in range(rep):
          for k in range(3):
              for R in BUCKETS:
                  nb = meta["bucket_counts"][k].get(R, 0)
                  if nb == 0:
                      continue
                  g = GMAP[R]
                  W = g * R
                  ngr = nb // g
                  row_base = row_bases[(k, R)]
                  # chunked staging loads: pslot (conv1) / rawT columns
                  ck_ps = max(1, 4096 // (W * 128))
                  ck_rw = max(1, 4096 // (g * 128))
                  ck = min(ck_ps, ck_rw) if k == 0 else ck_rw
                  for g0 in range(0, ngr, ck):
                      n_in = min(ck, ngr - g0)
                      if k == 0:
                          pch = pps.tile([4, ck * W * 128], bf16, tag="pslot")
                          nc.sync.dma_start(
                              out=pch[:, :n_in * W * 128]
                                  .rearrange("p (n w) -> p n w", w=W * 128),
                              in_=t_pslot[R].ap()[g0:g0 + n_in]
                                  .rearrange("n p w -> p n w"))
                      rch = praw.tile([Fs[k], ck * g * 128], bf16, tag="rawC")
                      cb = row_base + g0 * g * 128
                      nc.sync.dma_start(
                          out=rch[:, :n_in * g * 128],
                          in_=t_rawT[k].ap()[:, cb:cb + n_in * g * 128])
                      for gi in range(g0, g0 + n_in):
                          off = gi - g0
                          pst = (pch[:, off * W * 128:(off + 1) * W * 128]
                                 if k == 0 else None)
                          rawl = rch[:, off * g * 128:(off + 1) * g * 128]
                          do_group(k, R, gi, pst, rawl)

    nc.compile()
    return nc


# ==========================================================================
# Entry point
# ==========================================================================

def kernel(**inputs):
    import os
    meta, per_core_inputs, per_core_rowmaps, cores = _host_prep(inputs)
    nc = _build_kernel(meta,
                       use_lrelu=not os.environ.get("BREP_NO_LRELU"))

    in_maps = [dict(im) for im in per_core_inputs]

    import os
    if os.environ.get("BREP_SIM"):
        from concourse.bass_interp import CoreSim
        nc_sim = _build_kernel(meta, use_lrelu=False)  # interp can't exec Lrelu
        results = []
        for ci in range(C):
            sim = CoreSim(nc_sim, trace=False)
            for name, arr in in_maps[ci].items():
                sim.tensor(name)[:] = arr
            sim.simulate()
            results.append({"f3": np.array(sim.tensor("f3"))})
    else:
        res = run_bass_kernel_spmd(nc, in_maps, core_ids=list(range(C)))
        results = res.results

    NF = inputs["face_surfaces"].shape[0]
    out = np.empty((NF, H), np.float32)
    for ci, (r, c) in enumerate(zip(results, cores)):
        f3 = r["f3"]
        rm = per_core_rowmaps[ci][2]          # local face -> table row
        out[c["lo"]:c["hi"]] = f3[rm]
    return out
